# revision 30
# baseline (speedup 1.0000x reference)
"""A3LALoss forward on 8 TRN2 NeuronCores (Bass/Tile, data-parallel over batch).

Strategy (data-parallel, batch sharded 8 ways, 1024 rows/core):
  phase A: energy = -logsumexp(feats chunk) per expert; per-class energy sums
           via one-hot matmuls on the TensorEngine; AllReduce #1 ([3,1000]).
  phase B: per-class chain -> w (output weights), logadd = w*log(prior),
           exp_logadd = prior**w broadcast to 128 partitions (bf16).
  phase C: main memory-bound pass over expert_logits: exp on ScalarE (bf16
           out), fused multiply+row-reduce (affine_mul_reduce) against
           exp_logadd on VectorE -> row softmax denominators.
           W_yi = logits[b, target[b]] via indirect-DMA gathers (SWDGE),
           overlapped with the AllReduce flight.
  phase D: theta = arccos(W_yi/S) via odd polynomial; per-region theta means
           via TensorE matmuls + AllReduce #2; delta weights; local loss
           partial (one-hot segment sums again on TensorE); AllReduce #3.

Host python only shards inputs and precomputes *integer-input-derived* helper
tables (one-hot comparands, gather indices, region masks, class counts /
priors) — every float flop over the data tensors happens on the NeuronCores.
"""

import sys

for _p in ("/opt/trn_rl_repo", "/root/.axon_site/_ro/trn_rl_repo"):
    if _p not in sys.path:
        sys.path.insert(0, _p)

import numpy as np

import concourse.bacc as bacc
import concourse.bass as bass
import concourse.tile as tile
from concourse import bass_utils, mybir
from concourse.tile import add_dep_helper

F32 = mybir.dt.float32
BF16 = mybir.dt.bfloat16
I32 = mybir.dt.int32
AF = mybir.ActivationFunctionType
OP = mybir.AluOpType

# problem shape (hardcoded per spec)
B, C, E = 8192, 1000, 3
M = 8            # cores
BL = B // M      # 1024 rows per core
P = 128          # partitions
T = BL // P      # 8 b-tiles per core
S = 30.0
RG = [list(range(M))]
NCHUNK = 6       # main-pass DMA chunks (2 b-tiles x 1 expert each) -> 12


# ----------------------------------------------------------------------------
# host-side precompute (integer inputs only)
# ----------------------------------------------------------------------------
def _region_points(cls_np, num_experts):
    region_num = int(cls_np.sum()) // (num_experts - 1)
    srt = np.sort(cls_np)
    pts = []
    now = 0
    for v in srt:
        now += int(v)
        if now > region_num:
            pts.append(int(v))
            now = 0
    pts = list(reversed(pts))
    lr = []
    for i in range(len(pts)):
        right = int(cls_np.max()) if i == 0 else pts[i - 1]
        lr.append((pts[i], right))
    lr.append((0, pts[-1]))
    return lr


def host_precompute(target, cls_num_list):
    cls = np.asarray(cls_num_list).astype(np.int64)
    tgt = np.asarray(target).astype(np.int64)
    region = _region_points(cls, E)
    prior = (cls / cls.sum()).astype(np.float32)
    log_prior = np.log(prior).astype(np.float32)
    target_num = cls[tgt]
    masks = np.stack(
        [
            ((target_num > l) & (target_num <= r)).astype(np.float32)
            for (l, r) in region[: E - 1]
        ],
        axis=1,
    )  # [B, 2]
    counts = np.bincount(tgt, minlength=C).astype(np.float32)
    inv_counts = (1.0 / np.maximum(counts, 1.0)).astype(np.float32)
    present = (counts > 0).astype(np.float32)
    n_absent = float((counts == 0).sum())
    absent_neg = np.where(present > 0, 0.0, -1e30).astype(np.float32)
    rc = masks.sum(axis=0)
    inv_rc = (1.0 / np.maximum(rc, 1.0)).astype(np.float32)
    mkw = (masks * inv_rc[None, :]).astype(np.float32)

    rep3 = lambda v: np.repeat(v[None, :], 3, axis=0).astype(np.float32)
    return dict(
        masks=masks,
        mkw=mkw,
        iota=np.repeat(np.arange(C, dtype=np.float32)[None, :], 128, axis=0),
        logprior3=rep3(log_prior),
        invcnt3=rep3(inv_counts),
        absneg3=rep3(absent_neg),
        notpres3=rep3(1.0 - present),
        sel3=np.kron(np.eye(3, dtype=np.float32), np.ones((1, 128), np.float32)),
        n_absent=n_absent,
    )


def make_gidx(target_shard):
    """gidx[p, e*T+i] = flat index of logits[e, i*128+p, target] in the
    per-core [E*BL*C] flattened logits shard."""
    t = np.asarray(target_shard).astype(np.int64)
    gidx = np.zeros((P, E * T), dtype=np.int32)
    for e in range(E):
        for i in range(T):
            b = i * P + np.arange(P)
            gidx[:, e * T + i] = e * BL * C + b * C + t[b]
    return gidx


# ----------------------------------------------------------------------------
# device kernel
# ----------------------------------------------------------------------------
def build_module(n_absent, dbg=False):
    nc = bacc.Bacc("TRN2", target_bir_lowering=False, debug=False, num_devices=M)

    feats_d = nc.dram_tensor("feats", [BL, 192], F32, kind="ExternalInput")
    logits_d = nc.dram_tensor("logits", [E, BL, C], F32, kind="ExternalInput")
    tgt_d = nc.dram_tensor("tgt", [BL, 1], F32, kind="ExternalInput")
    gidx_d = nc.dram_tensor("gidx", [P, E * T], I32, kind="ExternalInput")
    mk_d = nc.dram_tensor("mk", [BL, 2], F32, kind="ExternalInput")
    mkw_d = nc.dram_tensor("mkw", [BL, 2], F32, kind="ExternalInput")
    iota_d = nc.dram_tensor("iota", [P, C], F32, kind="ExternalInput")
    logprior_d = nc.dram_tensor("logprior3", [3, C], F32, kind="ExternalInput")
    invcnt_d = nc.dram_tensor("invcnt3", [3, C], F32, kind="ExternalInput")
    absneg_d = nc.dram_tensor("absneg3", [3, C], F32, kind="ExternalInput")
    notpres_d = nc.dram_tensor("notpres3", [3, C], F32, kind="ExternalInput")
    sel3_d = nc.dram_tensor("sel3", [3, 3 * P], F32, kind="ExternalInput")
    out_d = nc.dram_tensor("out", [4, C], F32, kind="ExternalOutput")
    dbgP_d = nc.dram_tensor("dbgP", [P, 24 * 6], F32, kind="ExternalOutput") if dbg else None
    dbgC_d = nc.dram_tensor("dbgC", [3, C * 4], F32, kind="ExternalOutput") if dbg else None

    with tile.TileContext(nc) as tc:
        _build(nc, tc, n_absent, feats_d, logits_d, tgt_d, gidx_d, mk_d, mkw_d,
               iota_d, logprior_d, invcnt_d, absneg_d, notpres_d, sel3_d, out_d,
               dbgP_d, dbgC_d)

    nc.compile()
    return nc


def _build(nc, tc, n_absent, feats_d, logits_d, tgt_d, gidx_d, mk_d, mkw_d,
           iota_d, logprior_d, invcnt_d, absneg_d, notpres_d, sel3_d, out_d,
           dbgP_d=None, dbgC_d=None):
    from contextlib import ExitStack

    ctx = ExitStack()
    with ctx:
        cp = ctx.enter_context(tc.tile_pool(name="const", bufs=1))
        dp = ctx.enter_context(tc.tile_pool(name="dtiles", bufs=4))
        ep = ctx.enter_context(tc.tile_pool(name="extiles", bufs=12))
        sp = ctx.enter_context(tc.tile_pool(name="scratch", bufs=2))
        pp = ctx.enter_context(tc.tile_pool(name="psum", bufs=1, space="PSUM"))
        dr = ctx.enter_context(tc.tile_pool(name="dram", bufs=1, space="DRAM"))

        # ------------- tiny input DMAs -------------
        gidx = cp.tile([P, E * T], I32, tag="gidx")
        nc.sync.dma_start(out=gidx[:], in_=gidx_d.ap())
        ft = cp.tile([P, T * 192], F32, tag="ft")
        nc.sync.dma_start(
            out=ft[:].rearrange("p (t d) -> p t d", d=192),
            in_=feats_d.ap().rearrange("(t p) d -> p t d", p=P),
        )
        tg = cp.tile([P, T], F32, tag="tg")
        nc.sync.dma_start(
            out=tg[:].rearrange("p (t one) -> p t one", one=1),
            in_=tgt_d.ap().rearrange("(t p) one -> p t one", p=P),
        )
        mk = cp.tile([P, 2 * T], F32, tag="mk")
        nc.sync.dma_start(
            out=mk[:].rearrange("p (r t) -> p r t", r=2),
            in_=mk_d.ap().rearrange("(t p) r -> p r t", p=P),
        )
        mkw = cp.tile([P, 2 * T], F32, tag="mkw")
        nc.sync.dma_start(
            out=mkw[:].rearrange("p (r t) -> p r t", r=2),
            in_=mkw_d.ap().rearrange("(t p) r -> p r t", p=P),
        )
        iota_b = cp.tile([P, C], F32, tag="iota_b")
        nc.sync.dma_start(out=iota_b[:], in_=iota_d.ap())
        logprior3 = cp.tile([3, C], F32, tag="logprior3")
        nc.sync.dma_start(out=logprior3[:], in_=logprior_d.ap())
        invcnt3 = cp.tile([3, C], F32, tag="invcnt3")
        nc.sync.dma_start(out=invcnt3[:], in_=invcnt_d.ap())
        absneg3 = cp.tile([3, C], F32, tag="absneg3")
        nc.sync.dma_start(out=absneg3[:], in_=absneg_d.ap())
        notpres3 = cp.tile([3, C], F32, tag="notpres3")
        nc.sync.dma_start(out=notpres3[:], in_=notpres_d.ap())
        sel3 = cp.tile([3, 3 * P], F32, tag="sel3")
        nc.sync.dma_start(out=sel3[:], in_=sel3_d.ap())

        ones128 = cp.tile([P, 1], F32, tag="ones128")
        nc.vector.memset(ones128[:], 1.0)

        # ------------- one-hot tiles (VectorE) -------------
        oh = []
        oh_insts = []
        for i in range(T):
            t = cp.tile([P, C], BF16, tag=f"oh{i}")
            oi = nc.vector.tensor_scalar(
                out=t[:], in0=iota_b[:], scalar1=tg[:, i : i + 1],
                scalar2=None, op0=OP.is_equal,
            )
            oh.append(t)
            oh_insts.append(oi)

        # ------------- energies: one exp + one 3D reduce -------------
        # s24g layout: col = i*3 + e   (ft is [p, (t d)] with d = (e, 64))
        expft = cp.tile([P, T * 192], F32, tag="expft")
        nc.scalar.activation(out=expft[:], in_=ft[:], func=AF.Exp)
        s24g = cp.tile([P, 24], F32, tag="s24g")
        s24g_i = nc.vector.tensor_reduce(
            out=s24g[:].rearrange("p (g one) -> p g one", one=1),
            in_=expft[:].rearrange("p (g k) -> p g k", k=64),
            axis=mybir.AxisListType.X, op=OP.add,
        )
        add_dep_helper(s24g_i.ins, oh_insts[-1].ins, sync=False,
                       reason="energy reduce after oh gens on Vector")
        l24g = cp.tile([P, 24], F32, tag="l24g")
        nc.scalar.activation(out=l24g[:], in_=s24g[:], func=AF.Ln)
        # en24 keeps col = e*T + i layout (transpose the (i,e) grouping via AP)
        en24 = cp.tile([P, 24], BF16, tag="en24")
        nc.vector.tensor_scalar(
            out=en24[:].rearrange("p (e i) -> p i e", e=3),
            in0=l24g[:].rearrange("p (i e) -> p i e", e=3),
            scalar1=-1.0, scalar2=None, op0=OP.mult,
        )

        # ------------- segment-sum matmuls (TensorE) -------------
        psum_seg = pp.tile([3, C], F32, tag="psum_seg")
        en3 = en24[:].rearrange("p (e i) -> p e i", i=T)
        for i in range(T):
            for lo, hi in ((0, 512), (512, C)):
                nc.tensor.matmul(
                    out=psum_seg[:, lo:hi],
                    lhsT=en3[:, :, i],
                    rhs=oh[i][:, lo:hi],
                    start=(i == 0),
                    stop=(i == T - 1),
                )
        seg3 = cp.tile([3, C], F32, tag="seg3")
        nc.vector.tensor_copy(out=seg3[:], in_=psum_seg[:])

        # ------------- W_yi gathers + AllReduce #1 -------------
        logits_flat = logits_d.ap().rearrange("e b c -> (e b c) ()")
        wy24 = cp.tile([P, 24], F32, tag="wy24")

        def gather(col):
            return nc.gpsimd.indirect_dma_start(
                out=wy24[:, col : col + 1], out_offset=None, in_=logits_flat,
                in_offset=bass.IndirectOffsetOnAxis(ap=gidx[:, col : col + 1], axis=0),
            )

        # continuous gather stream on the Q7 (theta needs all 24 before the
        # merged collective can trigger)
        g_all = []
        gp = None
        for col in range(24):
            g = gather(col)
            if gp is not None:
                add_dep_helper(g.ins, gp.ins, sync=False, reason="gather chain")
            gp = g
            g_all.append(g)

        cc1_in = dr.tile([4, C], F32, tag="cc1_in")
        cc1_out = dr.tile([4, C], F32, tag="cc1_out")
        cc1_dma = nc.gpsimd.dma_start(out=cc1_in[:][0:3, :], in_=seg3[:])
        add_dep_helper(cc1_dma.ins, g_all[-1].ins, sync=False, reason="cc1_in after gathers")
        g_mid = g_all

        # ------------- main-pass DMA + exp, chunks 0..7 -------------
        # chunk k: expert e = k // (T//2), row pair q = k % (T//2)
        ex_tiles = [None] * (E * (T // 2))

        def emit_chunk_load(k):
            e, q = divmod(k, T // 2)
            dt = dp.tile([P, 2 * C], F32, tag="dtile")
            nc.sync.dma_start(
                out=dt[:].rearrange("p (h c) -> p h c", h=2),
                in_=logits_d.ap()[e, 2 * q * P : (2 * q + 2) * P, :].rearrange(
                    "(h p) c -> p h c", p=P
                ),
            )
            ex = ep.tile([P, 2 * C], BF16, tag="ex")
            nc.scalar.activation(out=ex[:], in_=dt[:], func=AF.Exp)
            ex_tiles[k] = ex

        for k in range(8):
            emit_chunk_load(k)

        # ------------- theta path (AR-independent): poly + region matmuls +
        # AR2 trigger, all queued before the AR1-dependent chain ------------
        # theta = pi/2 - x - x^3/6 - 3x^5/40,  x = wyi/S  (|x| <= ~0.17)
        x1 = cp.tile([P, 24], F32, tag="x1")
        nc.vector.tensor_scalar(out=x1[:], in0=wy24[:], scalar1=1.0 / S, scalar2=None, op0=OP.mult)
        x2 = cp.tile([P, 24], F32, tag="x2")
        nc.vector.tensor_tensor(out=x2[:], in0=x1[:], in1=x1[:], op=OP.mult)
        t3 = cp.tile([P, 24], F32, tag="t3")
        nc.vector.tensor_scalar(
            out=t3[:], in0=x2[:], scalar1=3.0 / 40.0, scalar2=1.0 / 6.0,
            op0=OP.mult, op1=OP.add,
        )
        t4 = cp.tile([P, 24], F32, tag="t4")
        nc.vector.tensor_tensor(out=t4[:], in0=t3[:], in1=x2[:], op=OP.mult)
        t5 = cp.tile([P, 24], F32, tag="t5")
        nc.vector.tensor_scalar(out=t5[:], in0=t4[:], scalar1=1.0, scalar2=None, op0=OP.add)
        t6 = cp.tile([P, 24], F32, tag="t6")
        nc.vector.tensor_tensor(out=t6[:], in0=t5[:], in1=x1[:], op=OP.mult)
        th24 = cp.tile([P, 24], F32, tag="th24")
        nc.vector.tensor_scalar(
            out=th24[:], in0=t6[:], scalar1=-1.0, scalar2=float(np.pi / 2),
            op0=OP.mult, op1=OP.add,
        )

        # region theta sums: [2, 3] = mkw^T @ theta  (accumulated over tiles)
        psum_th = pp.tile([2, 3], F32, tag="psum_th")
        mkw3 = mkw[:].rearrange("p (r i) -> p r i", r=2)
        th3 = th24[:].rearrange("p (e i) -> p e i", i=T)
        for i in range(T):
            nc.tensor.matmul(
                out=psum_th[:],
                lhsT=mkw3[:, :, i],
                rhs=th3[:, :, i],
                start=(i == 0),
                stop=(i == T - 1),
            )
        ths = cp.tile([2, 3], F32, tag="ths")
        nc.vector.tensor_copy(out=ths[:], in_=psum_th[:])

        # ths [2,3] -> row 3 of the merged collective input (flattened r*3+e)
        cc1b_dma = nc.gpsimd.dma_start(
            out=cc1_in[:][3:4, 0:6].rearrange("one (r e) -> r (one e)", r=2), in_=ths[:]
        )
        add_dep_helper(cc1b_dma.ins, g_mid[-1].ins, sync=False, reason="ths dma after gathers")
        ar1 = nc.gpsimd.collective_compute(
            "AllReduce", OP.add, replica_groups=RG,
            ins=[cc1_in[:].opt()], outs=[cc1_out[:].opt()],
        )

        # ------------- chain, VectorE part (stalls until merged AR back) ---
        asum3 = cp.tile([3, C], F32, tag="asum3")
        asum3_dma = nc.gpsimd.dma_start(out=asum3[:], in_=cc1_out[:][0:3, :])
        mu_row = cp.tile([1, 6], F32, tag="mu_row")
        nc.gpsimd.dma_start(out=mu_row[:], in_=cc1_out[:][3:4, 0:6])
        ones1r = cp.tile([1, P], F32, tag="ones1r")
        nc.vector.memset(ones1r[:], 1.0)
        psum_mu = pp.tile([P, 6], F32, tag="psum_mu")
        nc.tensor.matmul(out=psum_mu[:], lhsT=ones1r[:], rhs=mu_row[:], start=True, stop=True)
        mu_b = cp.tile([P, 6], F32, tag="mu_b")  # col = r*3 + e
        nc.vector.tensor_copy(out=mu_b[:], in_=psum_mu[:])

        no_absent = n_absent == 0.0
        means = cp.tile([3, C], F32, tag="means")
        nc.vector.tensor_tensor(out=means[:], in0=asum3[:], in1=invcnt3[:], op=OP.mult)
        e3p = cp.tile([3, C], F32, tag="e3p")
        sx = cp.tile([3, 1], F32, tag="sx")
        if no_absent:
            # calib = means in [-7, -2]: exp is safe without max-subtraction
            nc.scalar.activation(out=e3p[:], in_=means[:], func=AF.Exp, accum_out=sx[:])
            e3 = e3p
            sumex = sx
        else:
            masked = cp.tile([3, C], F32, tag="masked")
            nc.vector.tensor_tensor(out=masked[:], in0=means[:], in1=absneg3[:], op=OP.add)
            mx = cp.tile([3, 1], F32, tag="mx")
            nc.vector.tensor_reduce(out=mx[:], in_=masked[:], axis=mybir.AxisListType.X, op=OP.max)
            negmx = cp.tile([3, 1], F32, tag="negmx")
            nc.vector.tensor_scalar(out=negmx[:], in0=mx[:], scalar1=-1.0, scalar2=None, op0=OP.mult)
            nc.scalar.activation(
                out=e3p[:], in_=masked[:], func=AF.Exp, bias=negmx[:, 0:1], accum_out=sx[:]
            )
            e3 = cp.tile([3, C], F32, tag="e3")
            nc.vector.tensor_tensor(out=e3[:], in0=e3p[:], in1=notpres3[:], op=OP.add)
            sumex = cp.tile([3, 1], F32, tag="sumex")
            nc.vector.tensor_scalar(
                out=sumex[:], in0=sx[:], scalar1=float(n_absent), scalar2=None, op0=OP.add
            )
        rcp = cp.tile([3, 1], F32, tag="rcp")
        nc.vector.reciprocal(out=rcp[:], in_=sumex[:])
        w3 = cp.tile([3, C], F32, tag="w3")
        nc.vector.tensor_scalar(
            out=w3[:], in0=e3[:], scalar1=rcp[:, 0:1], scalar2=1.0,
            op0=OP.mult, op1=OP.add,
        )
        wlp3 = cp.tile([3, C], F32, tag="wlp3")
        nc.vector.tensor_tensor(out=wlp3[:], in0=w3[:], in1=logprior3[:], op=OP.mult)
        pc3 = cp.tile([3, C], F32, tag="pc3")
        nc.scalar.activation(out=pc3[:], in_=wlp3[:], func=AF.Exp)

        # exp_logadd broadcast via PE selection-matmul + ScalarE psum->sbuf
        # copies (keeps the Q7 free; ScalarE is idle at this point)
        elb_all = cp.tile([P, 3 * C], BF16, tag="elb_all")
        for e in range(E):
            psb = pp.tile([P, 1024], F32, tag="psum_bcast")
            for lo, hi in ((0, 512), (512, C)):
                nc.tensor.matmul(
                    out=psb[:, lo:hi],
                    lhsT=sel3[:, e * P : (e + 1) * P],
                    rhs=pc3[:, lo:hi],
                    start=True, stop=True,
                )
            nc.scalar.copy(out=elb_all[:, e * C : (e + 1) * C], in_=psb[:, 0:C])
        elb = [elb_all[:, e * C : (e + 1) * C] for e in range(E)]

        # ------------- main-pass DMA + exp, chunks 8..11 -------------
        for k in range(8, E * (T // 2)):
            emit_chunk_load(k)

        # ------------- rowsum AMRs (VectorE) -------------
        rs24 = cp.tile([P, 24], F32, tag="rs24")
        amr_list = []
        for k in range(E * (T // 2)):
            e, q = divmod(k, T // 2)
            ex = ex_tiles[k]
            for h in range(2):
                i = 2 * q + h
                col = e * T + i
                scr = sp.tile([P, C], BF16, tag="scr_rs", bufs=4)
                amr_list.append(nc.vector.affine_mul_reduce(
                    out=scr[:], accum_out=rs24[:, col : col + 1],
                    in0=ex[:, h * C : (h + 1) * C], in1=elb[e],
                    scale=1.0, bias=0.0,
                ))


        lse24 = cp.tile([P, 24], F32, tag="lse24")
        nc.scalar.activation(out=lse24[:], in_=rs24[:], func=AF.Ln)

        # delta -> fw = 1 + relu(delta)  (kept after the AMR stream on Vector)
        fw24 = cp.tile([P, 24], F32, tag="fw24")
        for e in range(E):
            th_e = th24[:, e * T : (e + 1) * T]
            a0 = sp.tile([P, T], F32, tag="a0")
            a0_i = nc.vector.tensor_scalar(
                out=a0[:], in0=th_e, scalar1=mu_b[:, e : e + 1], scalar2=None, op0=OP.subtract
            )

            p0 = sp.tile([P, T], F32, tag="p0")
            nc.vector.tensor_tensor(out=p0[:], in0=a0[:], in1=mk[:, 0:T], op=OP.mult)
            a1 = sp.tile([P, T], F32, tag="a1")
            nc.vector.tensor_scalar(
                out=a1[:], in0=th_e, scalar1=mu_b[:, 3 + e : 4 + e], scalar2=None, op0=OP.subtract
            )
            p1 = sp.tile([P, T], F32, tag="p1")
            nc.vector.tensor_tensor(out=p1[:], in0=a1[:], in1=mk[:, T : 2 * T], op=OP.mult)
            d = sp.tile([P, T], F32, tag="d")
            nc.vector.tensor_tensor(out=d[:], in0=p0[:], in1=p1[:], op=OP.add)
            nc.vector.tensor_scalar(
                out=fw24[:, e * T : (e + 1) * T], in0=d[:], scalar1=0.0, scalar2=1.0,
                op0=OP.max, op1=OP.add,
            )

        # z = wyi - lse ; accA[:, e] = sum_i fw_e * z_e
        zz = cp.tile([P, 24], F32, tag="zz")
        nc.vector.tensor_tensor(out=zz[:], in0=wy24[:], in1=lse24[:], op=OP.subtract)
        accA = cp.tile([P, 3], F32, tag="accA")
        for e in range(E):
            scrE = sp.tile([P, T], F32, tag="scrE")
            nc.vector.affine_mul_reduce(
                out=scrE[:], accum_out=accA[:, e : e + 1],
                in0=fw24[:, e * T : (e + 1) * T],
                in1=zz[:, e * T : (e + 1) * T],
                scale=1.0, bias=0.0,
            )

        # dsum[e, c] = sum_b onehot * fw  (TensorE) ; ga = sum_c dsum*logadd
        fw24b = cp.tile([P, 24], BF16, tag="fw24b")
        nc.vector.tensor_copy(out=fw24b[:], in_=fw24[:])
        psum_d = pp.tile([3, C], F32, tag="psum_seg")
        fw3 = fw24b[:].rearrange("p (e i) -> p e i", i=T)
        for i in range(T):
            for lo, hi in ((0, 512), (512, C)):
                nc.tensor.matmul(
                    out=psum_d[:, lo:hi],
                    lhsT=fw3[:, :, i],
                    rhs=oh[i][:, lo:hi],
                    start=(i == 0),
                    stop=(i == T - 1),
                )
        dsum3 = cp.tile([3, C], F32, tag="dsum3")
        nc.vector.tensor_copy(out=dsum3[:], in_=psum_d[:])
        gaS = cp.tile([3, 1], F32, tag="gaS")
        scr3C = cp.tile([3, C], F32, tag="scr3C")
        nc.vector.affine_mul_reduce(
            out=scr3C[:], accum_out=gaS[:], in0=dsum3[:], in1=wlp3[:],
            scale=1.0, bias=0.0,
        )

        # partition reductions via ones-matmul
        pl = pp.tile([1, 3], F32, tag="pl")
        nc.tensor.matmul(out=pl[:], lhsT=ones128[:], rhs=accA[:], start=True, stop=True)
        gaq = pp.tile([1, 1], F32, tag="gaq")
        nc.tensor.matmul(out=gaq[:], lhsT=ones128[0:3, :], rhs=gaS[:], start=True, stop=True)
        plS = cp.tile([1, 3], F32, tag="plS")
        nc.vector.tensor_copy(out=plS[:], in_=pl[:])
        gaqS = cp.tile([1, 1], F32, tag="gaqS")
        nc.vector.tensor_copy(out=gaqS[:], in_=gaq[:])
        l1 = cp.tile([1, 1], F32, tag="l1")
        nc.vector.tensor_reduce(out=l1[:], in_=plS[:], axis=mybir.AxisListType.X, op=OP.add)
        tot = cp.tile([1, 1], F32, tag="tot")
        nc.vector.tensor_tensor(out=tot[:], in0=l1[:], in1=gaqS[:], op=OP.add)
        part = cp.tile([1, 1], F32, tag="part")
        nc.vector.tensor_scalar(
            out=part[:], in0=tot[:], scalar1=-1.0 / B, scalar2=None, op0=OP.mult
        )

        # ------------- outputs (loss partial summed on host) -------------
        nc.sync.dma_start(out=out_d.ap()[0:1, 0:1], in_=part[:])
        nc.sync.dma_start(out=out_d.ap()[1:4, :], in_=w3[:])
        if dbgP_d is not None:
            for kk, tt_ in enumerate([s24g, en24, rs24, wy24, th24, fw24]):
                nc.sync.dma_start(out=dbgP_d.ap()[:, kk * 24 : (kk + 1) * 24], in_=tt_[:])
            for kk, tt_ in enumerate([seg3, asum3, w3, wlp3]):
                nc.sync.dma_start(out=dbgC_d.ap()[:, kk * C : (kk + 1) * C], in_=tt_[:])


# ----------------------------------------------------------------------------
# entry point
# ----------------------------------------------------------------------------
def make_in_maps(output_features, expert_logits, target, cls_num_list):
    pre = host_precompute(target, cls_num_list)
    feats = np.ascontiguousarray(np.asarray(output_features, dtype=np.float32))
    logits = np.asarray(expert_logits, dtype=np.float32)
    tgtf = np.asarray(target, dtype=np.float32)[:, None]
    target = np.asarray(target)

    in_maps = []
    for m in range(M):
        sl = slice(m * BL, (m + 1) * BL)
        in_maps.append(
            {
                "feats": np.ascontiguousarray(feats[sl]),
                "logits": np.ascontiguousarray(logits[:, sl, :]),
                "tgt": np.ascontiguousarray(tgtf[sl]),
                "gidx": make_gidx(target[sl]),
                "mk": np.ascontiguousarray(pre["masks"][sl]),
                "mkw": np.ascontiguousarray(pre["mkw"][sl]),
                "iota": pre["iota"],
                "logprior3": pre["logprior3"],
                "invcnt3": pre["invcnt3"],
                "absneg3": pre["absneg3"],
                "notpres3": pre["notpres3"],
                "sel3": pre["sel3"],
            }
        )
    return in_maps, pre


_CACHED = {}


def _ensure_ntff_hook():
    """The agent image's antenv lacks axon_hooks; synthesize it so
    run_bass_kernel_spmd(trace=True) can capture NTFF profiles."""
    try:
        from antenv import axon_hooks  # noqa: F401
        return
    except ImportError:
        pass
    import types

    import antenv

    mod = types.ModuleType("antenv.axon_hooks")
    _state = {"hook": None}
    mod.set_axon_ntff_profile_hook = lambda h: _state.__setitem__("hook", h)
    mod.get_axon_ntff_profile_hook = lambda: _state["hook"]
    sys.modules["antenv.axon_hooks"] = mod
    antenv.axon_hooks = mod
    try:
        from trn_agent_boot.trn_boot import _ntff_profile_via_ctypes

        hook = _ntff_profile_via_ctypes("/opt/axon/libaxon_pjrt.so")
        if hook is not None:
            mod.set_axon_ntff_profile_hook(hook)
    except Exception as e:  # pragma: no cover
        print("ntff hook setup failed:", e, file=sys.stderr)


def run(output_features, expert_logits, target, cls_num_list, trace=False, dbg=False, **kw):
    if trace:
        _ensure_ntff_hook()
    in_maps, pre = make_in_maps(output_features, expert_logits, target, cls_num_list)
    key = ("v7", pre["n_absent"], dbg)
    if key not in _CACHED:
        _CACHED[key] = build_module(pre["n_absent"], dbg=dbg)
    nc = _CACHED[key]
    res = bass_utils.run_bass_kernel_spmd(
        nc, in_maps, core_ids=list(range(M)), trace=trace, **kw
    )
    loss = np.float32(sum(r["out"][0, 0] for r in res.results))
    weights = np.asarray(res.results[0]["out"][1:4, :], dtype=np.float32)
    return (loss, weights), res


def kernel(output_features, expert_logits, target, cls_num_list):
    (loss, weights), _ = run(output_features, expert_logits, target, cls_num_list)
    return loss, weights


if __name__ == "__main__":
    import reference

    inputs = reference.setup_inputs()
    out = kernel(**{k: np.asarray(v) for k, v in inputs.items()})
    print("loss:", out[0])
    print("w[:, :5]:", out[1][:, :5])


# revision 31
# speedup vs baseline: 1.2089x; 1.2089x over previous
"""A3LALoss forward on 8 TRN2 NeuronCores (Bass/Tile, data-parallel over batch).

Strategy (data-parallel, batch sharded 8 ways, 1024 rows/core):
  phase A: energy = -logsumexp(feats chunk) per expert; per-class energy sums
           via one-hot matmuls on the TensorEngine; AllReduce #1 ([3,1000]).
  phase B: per-class chain -> w (output weights), logadd = w*log(prior),
           exp_logadd = prior**w broadcast to 128 partitions (bf16).
  phase C: main memory-bound pass over expert_logits: exp on ScalarE (bf16
           out), fused multiply+row-reduce (affine_mul_reduce) against
           exp_logadd on VectorE -> row softmax denominators.
           W_yi = logits[b, target[b]] via indirect-DMA gathers (SWDGE),
           overlapped with the AllReduce flight.
  phase D: theta = arccos(W_yi/S) via odd polynomial; per-region theta sums
           ride in row 3 of the same (merged) AllReduce; delta weights; local
           loss partial (one-hot segment sums again on TensorE) summed on the
           host across the 8 cores (the batch-mean unshard step).

Host python only shards inputs and precomputes *integer-input-derived* helper
tables (one-hot comparands, gather indices, region masks, class counts /
priors) — every float flop over the data tensors happens on the NeuronCores.
"""

import sys

for _p in ("/opt/trn_rl_repo", "/root/.axon_site/_ro/trn_rl_repo"):
    if _p not in sys.path:
        sys.path.insert(0, _p)

import numpy as np

import concourse.bacc as bacc
import concourse.bass as bass
import concourse.tile as tile
from concourse import bass_utils, mybir
from concourse.tile import add_dep_helper

F32 = mybir.dt.float32
BF16 = mybir.dt.bfloat16
I32 = mybir.dt.int32
AF = mybir.ActivationFunctionType
OP = mybir.AluOpType

# problem shape (hardcoded per spec)
B, C, E = 8192, 1000, 3
M = 8            # cores
BL = B // M      # 1024 rows per core
P = 128          # partitions
T = BL // P      # 8 b-tiles per core
S = 30.0
RG = [list(range(M))]
NCHUNK = 6       # main-pass DMA chunks (2 b-tiles x 1 expert each) -> 12


# ----------------------------------------------------------------------------
# host-side precompute (integer inputs only)
# ----------------------------------------------------------------------------
def _region_points(cls_np, num_experts):
    region_num = int(cls_np.sum()) // (num_experts - 1)
    srt = np.sort(cls_np)
    pts = []
    now = 0
    for v in srt:
        now += int(v)
        if now > region_num:
            pts.append(int(v))
            now = 0
    pts = list(reversed(pts))
    lr = []
    for i in range(len(pts)):
        right = int(cls_np.max()) if i == 0 else pts[i - 1]
        lr.append((pts[i], right))
    lr.append((0, pts[-1]))
    return lr


def host_precompute(target, cls_num_list):
    cls = np.asarray(cls_num_list).astype(np.int64)
    tgt = np.asarray(target).astype(np.int64)
    region = _region_points(cls, E)
    prior = (cls / cls.sum()).astype(np.float32)
    log_prior = np.log(prior).astype(np.float32)
    target_num = cls[tgt]
    masks = np.stack(
        [
            ((target_num > l) & (target_num <= r)).astype(np.float32)
            for (l, r) in region[: E - 1]
        ],
        axis=1,
    )  # [B, 2]
    counts = np.bincount(tgt, minlength=C).astype(np.float32)
    inv_counts = (1.0 / np.maximum(counts, 1.0)).astype(np.float32)
    present = (counts > 0).astype(np.float32)
    n_absent = float((counts == 0).sum())
    absent_neg = np.where(present > 0, 0.0, -1e30).astype(np.float32)
    rc = masks.sum(axis=0)
    inv_rc = (1.0 / np.maximum(rc, 1.0)).astype(np.float32)
    mkw = (masks * inv_rc[None, :]).astype(np.float32)

    rep3 = lambda v: np.repeat(v[None, :], 3, axis=0).astype(np.float32)
    return dict(
        masks=masks,
        mkw=mkw,
        iota=np.repeat(np.arange(C, dtype=np.float32)[None, :], 128, axis=0),
        logprior3=rep3(log_prior),
        invcnt3=rep3(inv_counts),
        absneg3=rep3(absent_neg),
        notpres3=rep3(1.0 - present),
        sel3=np.kron(np.eye(3, dtype=np.float32), np.ones((1, 128), np.float32)),
        n_absent=n_absent,
    )


def make_gidx(target_shard):
    """gidx[p, e*T+i] = flat index of logits[e, i*128+p, target] in the
    per-core [E*BL*C] flattened logits shard."""
    t = np.asarray(target_shard).astype(np.int64)
    gidx = np.zeros((P, E * T), dtype=np.int32)
    for e in range(E):
        for i in range(T):
            b = i * P + np.arange(P)
            gidx[:, e * T + i] = e * BL * C + b * C + t[b]
    return gidx


# ----------------------------------------------------------------------------
# device kernel
# ----------------------------------------------------------------------------
def build_module(n_absent, dbg=False):
    nc = bacc.Bacc("TRN2", target_bir_lowering=False, debug=False, num_devices=M)

    feats_d = nc.dram_tensor("feats", [BL, 192], F32, kind="ExternalInput")
    logits_d = nc.dram_tensor("logits", [E, BL, C], F32, kind="ExternalInput")
    tgt_d = nc.dram_tensor("tgt", [BL, 1], F32, kind="ExternalInput")
    gidx_d = nc.dram_tensor("gidx", [P, E * T], I32, kind="ExternalInput")
    mk_d = nc.dram_tensor("mk", [BL, 2], F32, kind="ExternalInput")
    mkw_d = nc.dram_tensor("mkw", [BL, 2], F32, kind="ExternalInput")
    iota_d = nc.dram_tensor("iota", [P, C], F32, kind="ExternalInput")
    logprior_d = nc.dram_tensor("logprior3", [3, C], F32, kind="ExternalInput")
    invcnt_d = nc.dram_tensor("invcnt3", [3, C], F32, kind="ExternalInput")
    absneg_d = nc.dram_tensor("absneg3", [3, C], F32, kind="ExternalInput")
    notpres_d = nc.dram_tensor("notpres3", [3, C], F32, kind="ExternalInput")
    sel3_d = nc.dram_tensor("sel3", [3, 3 * P], F32, kind="ExternalInput")
    out_d = nc.dram_tensor("out", [4, C], F32, kind="ExternalOutput")
    dbgP_d = nc.dram_tensor("dbgP", [P, 24 * 6], F32, kind="ExternalOutput") if dbg else None
    dbgC_d = nc.dram_tensor("dbgC", [3, C * 4], F32, kind="ExternalOutput") if dbg else None

    with tile.TileContext(nc) as tc:
        _build(nc, tc, n_absent, feats_d, logits_d, tgt_d, gidx_d, mk_d, mkw_d,
               iota_d, logprior_d, invcnt_d, absneg_d, notpres_d, sel3_d, out_d,
               dbgP_d, dbgC_d)

    nc.compile()
    return nc


def _build(nc, tc, n_absent, feats_d, logits_d, tgt_d, gidx_d, mk_d, mkw_d,
           iota_d, logprior_d, invcnt_d, absneg_d, notpres_d, sel3_d, out_d,
           dbgP_d=None, dbgC_d=None):
    from contextlib import ExitStack

    ctx = ExitStack()
    with ctx:
        cp = ctx.enter_context(tc.tile_pool(name="const", bufs=1))
        dp = ctx.enter_context(tc.tile_pool(name="dtiles", bufs=4))
        ep = ctx.enter_context(tc.tile_pool(name="extiles", bufs=12))
        sp = ctx.enter_context(tc.tile_pool(name="scratch", bufs=2))
        pp = ctx.enter_context(tc.tile_pool(name="psum", bufs=1, space="PSUM"))
        dr = ctx.enter_context(tc.tile_pool(name="dram", bufs=1, space="DRAM"))

        # ------------- tiny input DMAs -------------
        gidx = cp.tile([P, E * T], I32, tag="gidx")
        nc.sync.dma_start(out=gidx[:], in_=gidx_d.ap())
        ft = cp.tile([P, T * 192], F32, tag="ft")
        nc.sync.dma_start(
            out=ft[:].rearrange("p (t d) -> p t d", d=192),
            in_=feats_d.ap().rearrange("(t p) d -> p t d", p=P),
        )
        tg = cp.tile([P, T], F32, tag="tg")
        nc.sync.dma_start(
            out=tg[:].rearrange("p (t one) -> p t one", one=1),
            in_=tgt_d.ap().rearrange("(t p) one -> p t one", p=P),
        )
        mk = cp.tile([P, 2 * T], F32, tag="mk")
        nc.sync.dma_start(
            out=mk[:].rearrange("p (r t) -> p r t", r=2),
            in_=mk_d.ap().rearrange("(t p) r -> p r t", p=P),
        )
        mkw = cp.tile([P, 2 * T], F32, tag="mkw")
        nc.sync.dma_start(
            out=mkw[:].rearrange("p (r t) -> p r t", r=2),
            in_=mkw_d.ap().rearrange("(t p) r -> p r t", p=P),
        )
        iota_b = cp.tile([P, C], F32, tag="iota_b")
        nc.sync.dma_start(out=iota_b[:], in_=iota_d.ap())
        logprior3 = cp.tile([3, C], F32, tag="logprior3")
        nc.sync.dma_start(out=logprior3[:], in_=logprior_d.ap())
        invcnt3 = cp.tile([3, C], F32, tag="invcnt3")
        nc.sync.dma_start(out=invcnt3[:], in_=invcnt_d.ap())
        absneg3 = cp.tile([3, C], F32, tag="absneg3")
        nc.sync.dma_start(out=absneg3[:], in_=absneg_d.ap())
        notpres3 = cp.tile([3, C], F32, tag="notpres3")
        nc.sync.dma_start(out=notpres3[:], in_=notpres_d.ap())
        sel3 = cp.tile([3, 3 * P], F32, tag="sel3")
        nc.sync.dma_start(out=sel3[:], in_=sel3_d.ap())

        ones128 = cp.tile([P, 1], F32, tag="ones128")
        nc.vector.memset(ones128[:], 1.0)

        # ------------- one-hot tiles (VectorE) -------------
        oh = []
        oh_insts = []
        for i in range(T):
            t = cp.tile([P, C], BF16, tag=f"oh{i}")
            oi = nc.vector.tensor_scalar(
                out=t[:], in0=iota_b[:], scalar1=tg[:, i : i + 1],
                scalar2=None, op0=OP.is_equal,
            )
            oh.append(t)
            oh_insts.append(oi)

        # ------------- energies: one exp + one 3D reduce -------------
        # s24g layout: col = i*3 + e   (ft is [p, (t d)] with d = (e, 64))
        expft = cp.tile([P, T * 192], F32, tag="expft")
        nc.scalar.activation(out=expft[:], in_=ft[:], func=AF.Exp)
        s24g = cp.tile([P, 24], F32, tag="s24g")
        s24g_i = nc.vector.tensor_reduce(
            out=s24g[:].rearrange("p (g one) -> p g one", one=1),
            in_=expft[:].rearrange("p (g k) -> p g k", k=64),
            axis=mybir.AxisListType.X, op=OP.add,
        )
        add_dep_helper(s24g_i.ins, oh_insts[-1].ins, sync=False,
                       reason="energy reduce after oh gens on Vector")
        l24g = cp.tile([P, 24], F32, tag="l24g")
        nc.scalar.activation(out=l24g[:], in_=s24g[:], func=AF.Ln)
        # en24 keeps col = e*T + i layout (transpose the (i,e) grouping via AP)
        en24 = cp.tile([P, 24], BF16, tag="en24")
        nc.vector.tensor_scalar(
            out=en24[:].rearrange("p (e i) -> p i e", e=3),
            in0=l24g[:].rearrange("p (i e) -> p i e", e=3),
            scalar1=-1.0, scalar2=None, op0=OP.mult,
        )

        # ------------- segment-sum matmuls (TensorE) -------------
        psum_seg = pp.tile([3, C], F32, tag="psum_seg")
        en3 = en24[:].rearrange("p (e i) -> p e i", i=T)
        for i in range(T):
            for lo, hi in ((0, 512), (512, C)):
                nc.tensor.matmul(
                    out=psum_seg[:, lo:hi],
                    lhsT=en3[:, :, i],
                    rhs=oh[i][:, lo:hi],
                    start=(i == 0),
                    stop=(i == T - 1),
                )
        seg3 = cp.tile([3, C], F32, tag="seg3")
        nc.vector.tensor_copy(out=seg3[:], in_=psum_seg[:])

        # ------------- W_yi gathers + AllReduce #1 -------------
        logits_flat = logits_d.ap().rearrange("e b c -> (e b c) ()")
        wy24 = cp.tile([P, 24], F32, tag="wy24")

        def gather(col):
            return nc.gpsimd.indirect_dma_start(
                out=wy24[:, col : col + 1], out_offset=None, in_=logits_flat,
                in_offset=bass.IndirectOffsetOnAxis(ap=gidx[:, col : col + 1], axis=0),
            )

        # continuous gather stream on the Q7 (theta needs all 24 before the
        # merged collective can trigger)
        g_all = []
        gp = None
        for col in range(24):
            g = gather(col)
            if gp is not None:
                add_dep_helper(g.ins, gp.ins, sync=False, reason="gather chain")
            gp = g
            g_all.append(g)

        cc1_in = dr.tile([4, C], F32, tag="cc1_in")
        cc1_out = dr.tile([4, C], F32, tag="cc1_out")
        cc1_dma = nc.gpsimd.dma_start(out=cc1_in[:][0:3, :], in_=seg3[:])
        add_dep_helper(cc1_dma.ins, g_all[-1].ins, sync=False, reason="cc1_in after gathers")
        g_mid = g_all

        # ------------- main-pass DMA + exp, chunks 0..7 -------------
        # chunk k: expert e = k // (T//2), row pair q = k % (T//2)
        ex_tiles = [None] * (E * (T // 2))

        def emit_chunk_load(k):
            e, q = divmod(k, T // 2)
            dt = dp.tile([P, 2 * C], F32, tag="dtile")
            nc.sync.dma_start(
                out=dt[:].rearrange("p (h c) -> p h c", h=2),
                in_=logits_d.ap()[e, 2 * q * P : (2 * q + 2) * P, :].rearrange(
                    "(h p) c -> p h c", p=P
                ),
            )
            ex = ep.tile([P, 2 * C], BF16, tag="ex")
            nc.scalar.activation(out=ex[:], in_=dt[:], func=AF.Exp)
            ex_tiles[k] = ex

        for k in range(8):
            emit_chunk_load(k)

        # ------------- theta path (AR-independent): poly + region matmuls +
        # AR2 trigger, all queued before the AR1-dependent chain ------------
        # theta = pi/2 - x - x^3/6 - 3x^5/40,  x = wyi/S  (|x| <= ~0.17)
        x1 = cp.tile([P, 24], F32, tag="x1")
        nc.vector.tensor_scalar(out=x1[:], in0=wy24[:], scalar1=1.0 / S, scalar2=None, op0=OP.mult)
        x2 = cp.tile([P, 24], F32, tag="x2")
        nc.vector.tensor_tensor(out=x2[:], in0=x1[:], in1=x1[:], op=OP.mult)
        t3 = cp.tile([P, 24], F32, tag="t3")
        nc.vector.tensor_scalar(
            out=t3[:], in0=x2[:], scalar1=3.0 / 40.0, scalar2=1.0 / 6.0,
            op0=OP.mult, op1=OP.add,
        )
        t4 = cp.tile([P, 24], F32, tag="t4")
        nc.vector.tensor_tensor(out=t4[:], in0=t3[:], in1=x2[:], op=OP.mult)
        t5 = cp.tile([P, 24], F32, tag="t5")
        nc.vector.tensor_scalar(out=t5[:], in0=t4[:], scalar1=1.0, scalar2=None, op0=OP.add)
        t6 = cp.tile([P, 24], F32, tag="t6")
        nc.vector.tensor_tensor(out=t6[:], in0=t5[:], in1=x1[:], op=OP.mult)
        th24 = cp.tile([P, 24], F32, tag="th24")
        nc.vector.tensor_scalar(
            out=th24[:], in0=t6[:], scalar1=-1.0, scalar2=float(np.pi / 2),
            op0=OP.mult, op1=OP.add,
        )

        # region theta sums: [2, 3] = mkw^T @ theta  (accumulated over tiles)
        psum_th = pp.tile([2, 3], F32, tag="psum_th")
        mkw3 = mkw[:].rearrange("p (r i) -> p r i", r=2)
        th3 = th24[:].rearrange("p (e i) -> p e i", i=T)
        for i in range(T):
            nc.tensor.matmul(
                out=psum_th[:],
                lhsT=mkw3[:, :, i],
                rhs=th3[:, :, i],
                start=(i == 0),
                stop=(i == T - 1),
            )
        ths = cp.tile([2, 3], F32, tag="ths")
        nc.vector.tensor_copy(out=ths[:], in_=psum_th[:])

        # ths [2,3] -> row 3 of the merged collective input (flattened r*3+e)
        cc1b_dma = nc.gpsimd.dma_start(
            out=cc1_in[:][3:4, 0:6].rearrange("one (r e) -> r (one e)", r=2), in_=ths[:]
        )
        add_dep_helper(cc1b_dma.ins, g_mid[-1].ins, sync=False, reason="ths dma after gathers")
        ar1 = nc.gpsimd.collective_compute(
            "AllReduce", OP.add, replica_groups=RG,
            ins=[cc1_in[:].opt()], outs=[cc1_out[:].opt()],
        )

        # ------------- chain, VectorE part (stalls until merged AR back) ---
        asum3 = cp.tile([3, C], F32, tag="asum3")
        asum3_dma = nc.gpsimd.dma_start(out=asum3[:], in_=cc1_out[:][0:3, :])
        mu_row = cp.tile([1, 6], F32, tag="mu_row")
        nc.gpsimd.dma_start(out=mu_row[:], in_=cc1_out[:][3:4, 0:6])
        ones1r = cp.tile([1, P], F32, tag="ones1r")
        nc.vector.memset(ones1r[:], 1.0)
        psum_mu = pp.tile([P, 6], F32, tag="psum_mu")
        nc.tensor.matmul(out=psum_mu[:], lhsT=ones1r[:], rhs=mu_row[:], start=True, stop=True)
        mu_b = cp.tile([P, 6], F32, tag="mu_b")  # col = r*3 + e
        nc.vector.tensor_copy(out=mu_b[:], in_=psum_mu[:])

        no_absent = n_absent == 0.0
        means = cp.tile([3, C], F32, tag="means")
        nc.vector.tensor_tensor(out=means[:], in0=asum3[:], in1=invcnt3[:], op=OP.mult)
        e3p = cp.tile([3, C], F32, tag="e3p")
        sx = cp.tile([3, 1], F32, tag="sx")
        if no_absent:
            # calib = means in [-7, -2]: exp is safe without max-subtraction
            nc.scalar.activation(out=e3p[:], in_=means[:], func=AF.Exp, accum_out=sx[:])
            e3 = e3p
            sumex = sx
        else:
            masked = cp.tile([3, C], F32, tag="masked")
            nc.vector.tensor_tensor(out=masked[:], in0=means[:], in1=absneg3[:], op=OP.add)
            mx = cp.tile([3, 1], F32, tag="mx")
            nc.vector.tensor_reduce(out=mx[:], in_=masked[:], axis=mybir.AxisListType.X, op=OP.max)
            negmx = cp.tile([3, 1], F32, tag="negmx")
            nc.vector.tensor_scalar(out=negmx[:], in0=mx[:], scalar1=-1.0, scalar2=None, op0=OP.mult)
            nc.scalar.activation(
                out=e3p[:], in_=masked[:], func=AF.Exp, bias=negmx[:, 0:1], accum_out=sx[:]
            )
            e3 = cp.tile([3, C], F32, tag="e3")
            nc.vector.tensor_tensor(out=e3[:], in0=e3p[:], in1=notpres3[:], op=OP.add)
            sumex = cp.tile([3, 1], F32, tag="sumex")
            nc.vector.tensor_scalar(
                out=sumex[:], in0=sx[:], scalar1=float(n_absent), scalar2=None, op0=OP.add
            )
        rcp = cp.tile([3, 1], F32, tag="rcp")
        nc.vector.reciprocal(out=rcp[:], in_=sumex[:])
        w3 = cp.tile([3, C], F32, tag="w3")
        nc.vector.tensor_scalar(
            out=w3[:], in0=e3[:], scalar1=rcp[:, 0:1], scalar2=1.0,
            op0=OP.mult, op1=OP.add,
        )
        wlp3 = cp.tile([3, C], F32, tag="wlp3")
        nc.vector.tensor_tensor(out=wlp3[:], in0=w3[:], in1=logprior3[:], op=OP.mult)
        pc3 = cp.tile([3, C], F32, tag="pc3")
        nc.scalar.activation(out=pc3[:], in_=wlp3[:], func=AF.Exp)

        # exp_logadd broadcast via PE selection-matmul + ScalarE psum->sbuf
        # copies (keeps the Q7 free; ScalarE is idle at this point)
        elb_all = cp.tile([P, 3 * C], BF16, tag="elb_all")
        for e in range(E):
            psb = pp.tile([P, 1024], F32, tag="psum_bcast")
            for lo, hi in ((0, 512), (512, C)):
                nc.tensor.matmul(
                    out=psb[:, lo:hi],
                    lhsT=sel3[:, e * P : (e + 1) * P],
                    rhs=pc3[:, lo:hi],
                    start=True, stop=True,
                )
            nc.scalar.copy(out=elb_all[:, e * C : (e + 1) * C], in_=psb[:, 0:C])
        elb = [elb_all[:, e * C : (e + 1) * C] for e in range(E)]

        # ------------- main-pass DMA + exp, chunks 8..11 -------------
        for k in range(8, E * (T // 2)):
            emit_chunk_load(k)

        # ------------- rowsum AMRs (VectorE) -------------
        rs24 = cp.tile([P, 24], F32, tag="rs24")
        amr_list = []
        for k in range(E * (T // 2)):
            e, q = divmod(k, T // 2)
            ex = ex_tiles[k]
            for h in range(2):
                i = 2 * q + h
                col = e * T + i
                scr = sp.tile([P, C], BF16, tag="scr_rs", bufs=4)
                amr_list.append(nc.vector.affine_mul_reduce(
                    out=scr[:], accum_out=rs24[:, col : col + 1],
                    in0=ex[:, h * C : (h + 1) * C], in1=elb[e],
                    scale=1.0, bias=0.0,
                ))


        lse24 = cp.tile([P, 24], F32, tag="lse24")
        nc.scalar.activation(out=lse24[:], in_=rs24[:], func=AF.Ln)

        # delta -> fw = 1 + relu(delta)  (kept after the AMR stream on Vector)
        fw24 = cp.tile([P, 24], F32, tag="fw24")
        for e in range(E):
            th_e = th24[:, e * T : (e + 1) * T]
            a0 = sp.tile([P, T], F32, tag="a0")
            a0_i = nc.vector.tensor_scalar(
                out=a0[:], in0=th_e, scalar1=mu_b[:, e : e + 1], scalar2=None, op0=OP.subtract
            )

            p0 = sp.tile([P, T], F32, tag="p0")
            nc.vector.tensor_tensor(out=p0[:], in0=a0[:], in1=mk[:, 0:T], op=OP.mult)
            a1 = sp.tile([P, T], F32, tag="a1")
            nc.vector.tensor_scalar(
                out=a1[:], in0=th_e, scalar1=mu_b[:, 3 + e : 4 + e], scalar2=None, op0=OP.subtract
            )
            p1 = sp.tile([P, T], F32, tag="p1")
            nc.vector.tensor_tensor(out=p1[:], in0=a1[:], in1=mk[:, T : 2 * T], op=OP.mult)
            d = sp.tile([P, T], F32, tag="d")
            nc.vector.tensor_tensor(out=d[:], in0=p0[:], in1=p1[:], op=OP.add)
            nc.vector.tensor_scalar(
                out=fw24[:, e * T : (e + 1) * T], in0=d[:], scalar1=0.0, scalar2=1.0,
                op0=OP.max, op1=OP.add,
            )

        # z = wyi - lse ; accA[:, e] = sum_i fw_e * z_e
        zz = cp.tile([P, 24], F32, tag="zz")
        nc.vector.tensor_tensor(out=zz[:], in0=wy24[:], in1=lse24[:], op=OP.subtract)
        accA = cp.tile([P, 3], F32, tag="accA")
        for e in range(E):
            scrE = sp.tile([P, T], F32, tag="scrE")
            nc.vector.affine_mul_reduce(
                out=scrE[:], accum_out=accA[:, e : e + 1],
                in0=fw24[:, e * T : (e + 1) * T],
                in1=zz[:, e * T : (e + 1) * T],
                scale=1.0, bias=0.0,
            )

        # dsum[e, c] = sum_b onehot * fw  (TensorE) ; ga = sum_c dsum*logadd
        fw24b = cp.tile([P, 24], BF16, tag="fw24b")
        nc.vector.tensor_copy(out=fw24b[:], in_=fw24[:])
        psum_d = pp.tile([3, C], F32, tag="psum_seg")
        fw3 = fw24b[:].rearrange("p (e i) -> p e i", i=T)
        for i in range(T):
            for lo, hi in ((0, 512), (512, C)):
                nc.tensor.matmul(
                    out=psum_d[:, lo:hi],
                    lhsT=fw3[:, :, i],
                    rhs=oh[i][:, lo:hi],
                    start=(i == 0),
                    stop=(i == T - 1),
                )
        dsum3 = cp.tile([3, C], F32, tag="dsum3")
        nc.vector.tensor_copy(out=dsum3[:], in_=psum_d[:])
        gaS = cp.tile([3, 1], F32, tag="gaS")
        scr3C = cp.tile([3, C], F32, tag="scr3C")
        nc.vector.affine_mul_reduce(
            out=scr3C[:], accum_out=gaS[:], in0=dsum3[:], in1=wlp3[:],
            scale=1.0, bias=0.0,
        )

        # partition reductions via ones-matmul
        pl = pp.tile([1, 3], F32, tag="pl")
        nc.tensor.matmul(out=pl[:], lhsT=ones128[:], rhs=accA[:], start=True, stop=True)
        gaq = pp.tile([1, 1], F32, tag="gaq")
        nc.tensor.matmul(out=gaq[:], lhsT=ones128[0:3, :], rhs=gaS[:], start=True, stop=True)
        plS = cp.tile([1, 3], F32, tag="plS")
        nc.vector.tensor_copy(out=plS[:], in_=pl[:])
        gaqS = cp.tile([1, 1], F32, tag="gaqS")
        nc.vector.tensor_copy(out=gaqS[:], in_=gaq[:])
        l1 = cp.tile([1, 1], F32, tag="l1")
        nc.vector.tensor_reduce(out=l1[:], in_=plS[:], axis=mybir.AxisListType.X, op=OP.add)
        tot = cp.tile([1, 1], F32, tag="tot")
        nc.vector.tensor_tensor(out=tot[:], in0=l1[:], in1=gaqS[:], op=OP.add)
        part = cp.tile([1, 1], F32, tag="part")
        nc.vector.tensor_scalar(
            out=part[:], in0=tot[:], scalar1=-1.0 / B, scalar2=None, op0=OP.mult
        )

        # ------------- outputs (loss partial summed on host) -------------
        nc.sync.dma_start(out=out_d.ap()[0:1, 0:1], in_=part[:])
        nc.sync.dma_start(out=out_d.ap()[1:4, :], in_=w3[:])
        if dbgP_d is not None:
            for kk, tt_ in enumerate([s24g, en24, rs24, wy24, th24, fw24]):
                nc.sync.dma_start(out=dbgP_d.ap()[:, kk * 24 : (kk + 1) * 24], in_=tt_[:])
            for kk, tt_ in enumerate([seg3, asum3, w3, wlp3]):
                nc.sync.dma_start(out=dbgC_d.ap()[:, kk * C : (kk + 1) * C], in_=tt_[:])


# ----------------------------------------------------------------------------
# entry point
# ----------------------------------------------------------------------------
def make_in_maps(output_features, expert_logits, target, cls_num_list):
    pre = host_precompute(target, cls_num_list)
    feats = np.ascontiguousarray(np.asarray(output_features, dtype=np.float32))
    logits = np.asarray(expert_logits, dtype=np.float32)
    tgtf = np.asarray(target, dtype=np.float32)[:, None]
    target = np.asarray(target)

    in_maps = []
    for m in range(M):
        sl = slice(m * BL, (m + 1) * BL)
        in_maps.append(
            {
                "feats": np.ascontiguousarray(feats[sl]),
                "logits": np.ascontiguousarray(logits[:, sl, :]),
                "tgt": np.ascontiguousarray(tgtf[sl]),
                "gidx": make_gidx(target[sl]),
                "mk": np.ascontiguousarray(pre["masks"][sl]),
                "mkw": np.ascontiguousarray(pre["mkw"][sl]),
                "iota": pre["iota"],
                "logprior3": pre["logprior3"],
                "invcnt3": pre["invcnt3"],
                "absneg3": pre["absneg3"],
                "notpres3": pre["notpres3"],
                "sel3": pre["sel3"],
            }
        )
    return in_maps, pre


_CACHED = {}


def _ensure_ntff_hook():
    """The agent image's antenv lacks axon_hooks; synthesize it so
    run_bass_kernel_spmd(trace=True) can capture NTFF profiles."""
    try:
        from antenv import axon_hooks  # noqa: F401
        return
    except ImportError:
        pass
    import types

    import antenv

    mod = types.ModuleType("antenv.axon_hooks")
    _state = {"hook": None}
    mod.set_axon_ntff_profile_hook = lambda h: _state.__setitem__("hook", h)
    mod.get_axon_ntff_profile_hook = lambda: _state["hook"]
    sys.modules["antenv.axon_hooks"] = mod
    antenv.axon_hooks = mod
    try:
        from trn_agent_boot.trn_boot import _ntff_profile_via_ctypes

        hook = _ntff_profile_via_ctypes("/opt/axon/libaxon_pjrt.so")
        if hook is not None:
            mod.set_axon_ntff_profile_hook(hook)
    except Exception as e:  # pragma: no cover
        print("ntff hook setup failed:", e, file=sys.stderr)


def run(output_features, expert_logits, target, cls_num_list, trace=False, dbg=False, **kw):
    if trace:
        _ensure_ntff_hook()
    in_maps, pre = make_in_maps(output_features, expert_logits, target, cls_num_list)
    key = ("v7", pre["n_absent"], dbg)
    if key not in _CACHED:
        _CACHED[key] = build_module(pre["n_absent"], dbg=dbg)
    nc = _CACHED[key]
    last_err = None
    for attempt in range(3):
        try:
            res = bass_utils.run_bass_kernel_spmd(
                nc, in_maps, core_ids=list(range(M)), trace=trace, **kw
            )
            break
        except Exception as e:  # transient NRT_EXEC_UNIT_UNRECOVERABLE on this stack
            last_err = e
            import time as _time

            print(f"run attempt {attempt} failed ({e}); retrying", file=sys.stderr)
            _time.sleep(2.0)
    else:
        raise last_err
    loss = np.float32(sum(r["out"][0, 0] for r in res.results))
    weights = np.asarray(res.results[0]["out"][1:4, :], dtype=np.float32)
    return (loss, weights), res


def kernel(output_features, expert_logits, target, cls_num_list):
    (loss, weights), _ = run(output_features, expert_logits, target, cls_num_list)
    return loss, weights


if __name__ == "__main__":
    import reference

    inputs = reference.setup_inputs()
    out = kernel(**{k: np.asarray(v) for k, v in inputs.items()})
    print("loss:", out[0])
    print("w[:, :5]:", out[1][:, :5])
